# revision 20
# baseline (speedup 1.0000x reference)
"""TF-Mamba block kernel for Trainium2 (8 NeuronCores, SPMD).

The block is two sequential mamba_bidir+linear stages (over time, then over
frequency). Both stages are the same computation on a reshaped token grid,
so ONE bass program ("half-block") is compiled and launched twice; the host
reshards between launches (numpy transpose).

Per launch each core processes 16 sequences x 128 steps for both scan
directions. Token layout on-chip: [channel-partitions, tau*16 + s] where
tau is the scan position and s the sequence. The selective scan runs on the
DVE tensor_tensor_scan instruction with lanes = (s, d_lo) and free dim =
chained (n, tau) segments separated by zeroed reset columns.
"""
import sys

sys.path.insert(0, "/opt/trn_rl_repo")

import numpy as np

import concourse.bass as bass
import concourse.tile as tile
from concourse import mybir
from concourse._compat import with_exitstack

F32 = mybir.dt.float32
BF16 = mybir.dt.bfloat16
AF = mybir.ActivationFunctionType
OP = mybir.AluOpType
EPS = 1e-5

C = 64        # channels
S = 16        # sequences per core per direction
L = 128       # scan length (tau)
T = L * S     # tokens per direction = 2048
DM = 256      # inner model dim (expand*C)
N = 16        # state size
NSEG = L + 1  # scan segment incl. reset column
NCORES = 8

_DIR_PARAMS = [
    ("wz", [C, DM]),       # lhsT for z projection
    ("wca", [128, DM]),    # conv+in_proj fused lhsT, K-tile 0 (k=0,1)
    ("wcb", [128, DM]),    # K-tile 1 (k=2,3)
    ("cbias", [128, 2]),   # conv bias, column per d-tile
    ("wxa", [128, 36]),    # x_proj lhsT K-tile 0
    ("wxb", [128, 36]),
    ("wdt", [4, DM]),
    ("bdt", [128, 2]),
    ("arep", [128, 32 * N]),  # A replicated: [s*8+dlo, dhi*16+n]
    ("dpar", [128, 2]),
    ("wpa", [128, C]),     # out_proj lhsT K-tile 0
    ("wpb", [128, C]),
]


def split_excess_waits(nc, maxw=1):
    """walrus setupSyncWait rejects instructions carrying more than ~2 sync
    waits; hoist the excess onto preceding NoOps on the same engine."""
    ctr = [0]

    def mknop(engine, waits):
        ctr[0] += 1
        nop = mybir.InstNoOp(name=f"waitfix_nop_{ctr[0]}", ins=[], outs=[])
        nop.engine = engine
        nop.sync_info = mybir.SyncInfo(on_wait=list(waits), on_update=[])
        return nop

    for f in nc.m.functions:
        for b in f.blocks:
            out = []
            changed = False
            for inst in b.instructions:
                si = inst.sync_info
                if si is not None and si.on_wait and len(si.on_wait) > maxw:
                    waits = list(si.on_wait)
                    keep, rest = waits[:maxw], waits[maxw:]
                    for k in range(0, len(rest), maxw):
                        out.append(mknop(inst.engine, rest[k:k + maxw]))
                    si.on_wait = keep
                    changed = True
                out.append(inst)
            if changed:
                b.instructions = out


def _seg(t_):
    """[128, 16*129] buffer -> [128, n, tau] view skipping reset columns."""
    return t_[:].rearrange("p (n q) -> p n q", n=N, q=NSEG)[:, :, 1:NSEG]


def _pads(t_):
    return t_[:].rearrange("p (n q) -> p n q", n=N, q=NSEG)[:, :, 0:1]


def _smaj(ap, s=S, t=L):
    """[p, T tau-major] -> [p, s, tau] view."""
    return ap.rearrange("p (t s) -> p t s", t=t, s=s).transpose([0, 2, 1])


def _sview(ap, s=S, t=L):
    """[p, T s-major] -> [p, s, tau] view."""
    return ap.rearrange("p (s t) -> p s t", s=s, t=t)


def _pslq(ap, q):
    """quarter partition window [32q, 32q+32): lanes p = dlo*16 + s."""
    return ap[q * 32:(q + 1) * 32]


SIM_COMPAT = False


@with_exitstack
def _half_block(ctx, tc):
    nc = tc.nc

    def act_silu(out, in_, bias=0.0, tmp_pool=None):
        if not SIM_COMPAT:
            nc.scalar.activation(out, in_, AF.Silu, bias=bias)
            return
        t1 = tmp_pool.tile(list(in_.shape), F32, tag="silu_t1", name="silu_t1", bufs=1)
        t2 = tmp_pool.tile(list(in_.shape), F32, tag="silu_t2", name="silu_t2", bufs=1)
        nc.scalar.activation(t1[:], in_, AF.Sigmoid, bias=bias)
        nc.scalar.activation(t2[:], in_, AF.Identity, bias=bias)
        nc.vector.tensor_tensor(out, t1[:], t2[:], OP.mult)

    def act_softplus(out, in_, bias, tmp_pool=None):
        # softplus(x+b) = ln(1 + exp(x+b)); Exp and Ln share one ACT
        # table set (natural_log_exp_and_others), as does the scan's Exp.
        t1 = tmp_pool.tile(list(in_.shape), F32, tag="silu_t1",
                           name="silu_t1", bufs=1)
        nc.scalar.activation(t1[:], in_, AF.Exp, bias=bias)
        nc.scalar.activation(out, t1[:], AF.Ln, bias=1.0)
    xin = nc.declare_dram_parameter("xin", [C, T], F32, isOutput=False)
    xout = nc.declare_dram_parameter("xout", [C, T], F32, isOutput=True)
    dirp = []
    for d in range(2):
        dirp.append({
            k: nc.declare_dram_parameter(f"{k}{d}", shp, F32, isOutput=False)
            for k, shp in _DIR_PARAMS
        })
    linw = nc.declare_dram_parameter("linw", [2 * C, C], F32, isOutput=False)
    linb = nc.declare_dram_parameter("linb", [C, 1], F32, isOutput=False)

    wp = ctx.enter_context(tc.tile_pool(name="weights", bufs=1))
    pers = ctx.enter_context(tc.tile_pool(name="pers", bufs=1))
    work = ctx.enter_context(tc.tile_pool(name="work", bufs=1))
    chn = ctx.enter_context(tc.tile_pool(name="chn", bufs=2))
    pp = ctx.enter_context(tc.tile_pool(name="pp", bufs=4, space="PSUM"))
    dram = ctx.enter_context(tc.tile_pool(name="dram", bufs=1, space="DRAM"))

    W = []
    for d in range(2):
        w = {}
        for k, shp in _DIR_PARAMS:
            t_ = wp.tile(shp, F32, tag=f"{k}{d}")
            nc.sync.dma_start(t_[:], dirp[d][k][:])
            w[k] = t_
        W.append(w)
    linw_t = wp.tile([2 * C, C], F32, tag="linw", name="linw")
    nc.sync.dma_start(linw_t[:], linw[:])
    linb_t = wp.tile([C, 1], F32, tag="linb", name="linb")
    nc.sync.dma_start(linb_t[:], linb[:])

    ones_c = wp.tile([C, 1], F32, tag="ones_c", name="ones_c")
    nc.gpsimd.memset(ones_c[:], 1.0)
    ones_1 = wp.tile([1, C], F32, tag="ones_1", name="ones_1")
    nc.gpsimd.memset(ones_1[:], 1.0)
    epsb = wp.tile([1, 1], F32, tag="epsb", name="epsb")
    nc.gpsimd.memset(epsb[:], EPS)

    # ---- shared: load X, rms scale, xn ----
    X = pers.tile([C, T], F32, tag="X", name="X")
    nc.sync.dma_start(X[:], xin[:])
    xsq = work.tile([C, T], F32, tag="scrA", name="scrA")
    nc.scalar.activation(xsq[:], X[:], AF.Square)
    rowstats = work.tile([1, T], F32, tag="scrB", name="scrB")
    for ch in range(4):
        sl = slice(ch * 512, (ch + 1) * 512)
        ps1 = pp.tile([1, 512], F32, tag="ps", name="ps")
        nc.tensor.matmul(ps1[:], ones_c[:], xsq[:, sl], start=True,
                         stop=True)
        nc.scalar.activation(rowstats[:, sl], ps1[:], AF.Sqrt, scale=1.0 / C,
                             bias=epsb[:])
    nc.vector.reciprocal(rowstats[:], rowstats[:])
    rinv = rowstats
    xn = pers.tile([C, T], F32, tag="xn", name="xn")
    for ch in range(4):
        sl = slice(ch * 512, (ch + 1) * 512)
        ps2 = pp.tile([C, 512], F32, tag="ps", name="ps")
        nc.tensor.matmul(ps2[:], ones_1[:], rinv[:, sl], start=True,
                         stop=True)
        nc.vector.tensor_tensor(xn[:, sl], X[:, sl], ps2[:], OP.mult)

    cat = pers.tile([2 * C, T], F32, tag="cat", name="cat")
    xn3 = xn[:].rearrange("p (t s) -> p t s", t=L, s=S)

    for d in range(2):
        fwd = (d == 0)
        w = W[d]
        # ---- conv stacks (shift in tau; bwd reads xn reversed in tau) ----
        stks = [work.tile([128, T], F32, tag=f"stk{i}", name=f"stk{i}") for i in range(2)]
        for k in range(4):
            dst = stks[k // 2][(k % 2) * C:(k % 2) * C + C, :]
            dst3 = dst.rearrange("p (t s) -> p t s", t=L, s=S)
            pad = 3 - k
            src3 = xn3 if fwd else xn3[:, ::-1, :]
            if pad > 0:
                nc.gpsimd.memset(dst3[:, 0:pad, :], 0.0)
                nc.vector.tensor_copy(dst3[:, pad:L, :], src3[:, 0:L - pad, :])
            else:
                nc.vector.tensor_copy(dst3, src3)

        # ---- z projection + silu (natural tau order for both dirs) ----
        zsil = work.tile([128, 2 * T], BF16, tag="zsil", name="zsil")
        for mt in range(2):
            for ch in range(4):
                sl = slice(ch * 512, (ch + 1) * 512)
                osl = slice(mt * T + ch * 512, mt * T + (ch + 1) * 512)
                ps = pp.tile([128, 512], F32, tag="ps", name="ps")
                nc.tensor.matmul(ps[:],
                                 w["wz"][:, mt * 128:mt * 128 + 128],
                                 xn[:, sl], start=True, stop=True)
                act_silu(zsil[:, osl], ps[:], tmp_pool=pp)

        # ---- fused conv + in_proj-xc + bias + silu ----
        xc = work.tile([128, 2 * T], F32, tag="xc", name="xc")
        for mt in range(2):
            msl = slice(mt * 128, mt * 128 + 128)
            for ch in range(4):
                sl = slice(ch * 512, (ch + 1) * 512)
                osl = slice(mt * T + ch * 512, mt * T + (ch + 1) * 512)
                ps = pp.tile([128, 512], F32, tag="ps", name="ps")
                nc.tensor.matmul(ps[:], w["wca"][:, msl], stks[0][:, sl],
                                 start=True, stop=False)
                nc.tensor.matmul(ps[:], w["wcb"][:, msl], stks[1][:, sl],
                                 start=False, stop=True)
                act_silu(xc[:, osl], ps[:], bias=w["cbias"][:, mt:mt + 1],
                         tmp_pool=pp)

        # ---- x_proj -> dbl [36, T] ----
        dbl = work.tile([36, T], F32, tag="dbl_brep", name="dbl_brep")
        for ch in range(4):
            sl = slice(ch * 512, (ch + 1) * 512)
            ps = pp.tile([36, 512], F32, tag="ps", name="ps")
            nc.tensor.matmul(ps[:], w["wxa"][:],
                             xc[:, sl], start=True, stop=False)
            nc.tensor.matmul(ps[:], w["wxb"][:],
                             xc[:, T + ch * 512:T + (ch + 1) * 512],
                             start=False, stop=True)
            nc.scalar.copy(dbl[:, sl], ps[:])

        # ---- dt = softplus(dt_proj + bias), written s-major ----
        dblr = wp.tile([4, T], F32, tag="dblr", name="dblr")
        nc.vector.tensor_copy(dblr[:], dbl[32:36, :])
        dt_mm = work.tile([128, 2 * T], F32, tag="dtmm_ymm", name="dtmm_ymm")
        for mt in range(2):
            msl = slice(mt * 128, mt * 128 + 128)
            for ch in range(4):
                ps = pp.tile([128, 512], F32, tag="ps", name="ps")
                nc.tensor.matmul(ps[:], w["wdt"][:, msl],
                                 dblr[0:4, ch * 512:(ch + 1) * 512],
                                 start=True, stop=True)
                psr = ps[:].rearrange("p (t s) -> p t s", t=32, s=S)
                dtv = dt_mm[:, mt * T:(mt + 1) * T]
                dtv = dtv.rearrange("p (s t) -> p s t", s=S, t=L)
                dtv = dtv[:, :, ch * 32:(ch + 1) * 32].transpose([0, 2, 1])
                act_softplus(dtv, psr, w["bdt"][:, mt:mt + 1], tmp_pool=pp)

        # ---- u = dt * xc (s-major) ----
        u_mm = work.tile([128, 2 * T], F32, tag="umm_g2", name="umm_g2")
        for mt in range(2):
            nc.vector.tensor_tensor(
                _sview(u_mm[:, mt * T:(mt + 1) * T]),
                _sview(dt_mm[:, mt * T:(mt + 1) * T]),
                _smaj(xc[:, mt * T:(mt + 1) * T]), OP.mult)

        # ---- B/C staging s-major [32, T] ----
        bcs = work.tile([32, T], BF16, tag="bcs_crep", name="bcs_crep")
        nc.vector.tensor_copy(_sview(bcs[:]), _smaj(dbl[0:32, :]))

        # ---- shuffles through DRAM ----
        dt_d = dram.tile([DM, T], F32, tag="dt_d", name="dt_d")
        u_d = dram.tile([DM, T], F32, tag="u_d", name="u_d")
        bc_d = dram.tile([128, 32 * L], BF16, tag="bc_d", name="bc_d")
        for mt in range(2):
            msl = slice(mt * 128, (mt + 1) * 128)
            nc.sync.dma_start(dt_d[msl, :], dt_mm[:, mt * T:(mt + 1) * T])
            nc.sync.dma_start(u_d[msl, :], u_mm[:, mt * T:(mt + 1) * T])
        # bcs rows (n') scatter to bc_d[dlo*16+s][n'][tau], replicated
        # over dlo so the scan-land read is a plain [128, :] copy
        for dlo in range(8):
            nc.sync.dma_start(
                bc_d[dlo * S:(dlo + 1) * S, :].rearrange(
                    "s (n t) -> n s t", n=32, t=L), bcs[:])

        dtS = work.tile([128, 32 * L], F32, tag="stk0", name="dtS")
        uS = work.tile([128, 32 * L], F32, tag="stk1", name="uS")
        brep = work.tile([128, N * L], BF16, tag="dbl_brep", name="brep")
        crep = work.tile([128, N * L], BF16, tag="bcs_crep", name="crep")
        dtdv = dt_d[:].rearrange("(dhi dlo) f -> dhi dlo f", dhi=32, dlo=8)
        udv = u_d[:].rearrange("(dhi dlo) f -> dhi dlo f", dhi=32, dlo=8)
        for dhi in range(32):
            fsl = slice(dhi * L, (dhi + 1) * L)
            nc.sync.dma_start(dtS[:, fsl],
                              dtdv[dhi].rearrange("dlo (s t) -> (dlo s) t",
                                                  s=S, t=L))
            nc.sync.dma_start(uS[:, fsl],
                              udv[dhi].rearrange("dlo (s t) -> (dlo s) t",
                                                 s=S, t=L))
        nc.sync.dma_start(brep[:], bc_d[:, 0:N * L])
        nc.sync.dma_start(crep[:], bc_d[:, N * L:2 * N * L])
        brep3 = brep[:].rearrange("p (n t) -> p n t", n=N, t=L)
        crep3 = crep[:].rearrange("p (n t) -> p n t", n=N, t=L)

        # ---- scan over d_hi chunks ----
        yS = work.tile([128, 32 * L], F32, tag="scrA", name="scrA")
        for g in range(32):
            gsl = slice(g * L, (g + 1) * L)
            dA = chn.tile([128, N * NSEG], F32, tag="dA", name="dA")
            ub = chn.tile([128, N * NSEG], BF16, tag="ub", name="ub")
            h = chn.tile([128, N * NSEG], BF16, tag="h", name="h")
            yt = chn.tile([128, N * L], BF16, tag="yt", name="yt", bufs=1)
            nc.gpsimd.memset(_pads(dA), 0.0)
            nc.gpsimd.memset(_pads(ub), 0.0)
            dts_b = dtS[:, gsl].unsqueeze(1).broadcast_to([128, N, L])
            ar_b = w["arep"][:, g * N:(g + 1) * N].unsqueeze(2).broadcast_to(
                [128, N, L])
            nc.vector.tensor_tensor(_seg(dA), dts_b, ar_b, OP.mult)
            nc.scalar.activation(_seg(dA), _seg(dA), AF.Exp)
            us_b = uS[:, gsl].unsqueeze(1).broadcast_to([128, N, L])
            nc.vector.tensor_tensor(_seg(ub), us_b, brep3, OP.mult)
            nc.vector.tensor_tensor_scan(h[:], dA[:], ub[:], 0.0,
                                         OP.mult, OP.add)
            ytv = yt[:].rearrange("p (t n) -> p t n", t=L, n=N)
            nc.vector.tensor_tensor(ytv.transpose([0, 2, 1]), _seg(h),
                                    crep3, OP.mult)
            nc.vector.tensor_reduce(yS[:, gsl], ytv, mybir.AxisListType.X,
                                    OP.add)

        # ---- y back to [d, (s,t)] ----
        y_d = dram.tile([DM, T], F32, tag="y_d", name="y_d")
        ydv = y_d[:].rearrange("(dhi dlo) f -> dhi dlo f", dhi=32, dlo=8)
        for dhi in range(32):
            nc.sync.dma_start(
                ydv[dhi].rearrange("dlo (s t) -> (dlo s) t", s=S, t=L),
                yS[:, dhi * L:(dhi + 1) * L])
        y_mm = work.tile([128, 2 * T], F32, tag="dtmm_ymm", name="dtmm_ymm")
        for mt in range(2):
            nc.sync.dma_start(y_mm[:, mt * T:(mt + 1) * T],
                              y_d[mt * 128:(mt + 1) * 128, :])

        # ---- gating: g2 = (y + D*xc) * silu(z), all s-major ----
        g2 = work.tile([128, 2 * T], F32, tag="umm_g2", name="umm_g2")
        for mt in range(2):
            g1 = work.tile([128, T], F32, tag="scrB", name="scrB")
            nc.vector.scalar_tensor_tensor(
                _sview(g1[:]), _smaj(xc[:, mt * T:(mt + 1) * T]),
                w["dpar"][:, mt:mt + 1],
                _sview(y_mm[:, mt * T:(mt + 1) * T]), OP.mult, OP.add)
            zv = _smaj(zsil[:, mt * T:(mt + 1) * T])
            if not fwd:
                zv = zv[:, :, ::-1]
            nc.vector.tensor_tensor(_sview(g2[:, mt * T:(mt + 1) * T]),
                                    _sview(g1[:]), zv, OP.mult)

        # ---- out_proj + residual into cat rows ----
        catd = _sview(cat[d * C:d * C + C, :])
        Xv = _smaj(X[:])
        for ch in range(4):
            sl = slice(ch * 512, (ch + 1) * 512)
            ssl = slice(ch * 4, (ch + 1) * 4)
            ps = pp.tile([C, 512], F32, tag="ps", name="ps")
            nc.tensor.matmul(ps[:], w["wpa"][:], g2[:, sl],
                             start=True, stop=False)
            nc.tensor.matmul(ps[:], w["wpb"][:],
                             g2[:, T + ch * 512:T + (ch + 1) * 512],
                             start=False, stop=True)
            psv = ps[:].rearrange("p (s t) -> p s t", s=4, t=L)
            if fwd:
                nc.vector.tensor_tensor(catd[:, ssl, :], psv, Xv[:, ssl, :],
                                        OP.add)
            else:
                nc.vector.tensor_tensor(catd[:, ssl, :], psv[:, :, ::-1],
                                        Xv[:, ssl, :], OP.add)

    # ---- lin + bias + residual -> xout (cat is s-major natural-t) ----
    xo = work.tile([C, T], F32, tag="scrB", name="scrB")
    for ch in range(4):
        sl = slice(ch * 512, (ch + 1) * 512)
        ssl = slice(ch * 4, (ch + 1) * 4)
        ps = pp.tile([C, 512], F32, tag="ps", name="ps")
        nc.tensor.matmul(ps[:], linw_t[:], cat[:, sl], start=True,
                         stop=True)
        psv = ps[:].rearrange("p (s t) -> p s t", s=4, t=L)
        nc.vector.scalar_tensor_tensor(_smaj(xo[:])[:, ssl, :], psv,
                                       linb_t[:], _smaj(X[:])[:, ssl, :],
                                       OP.add, OP.add)
    nc.sync.dma_start(xout[:], xo[:])


def build_program(waitfix=True, sim_compat=False):
    global SIM_COMPAT
    SIM_COMPAT = sim_compat
    nc = bass.Bass(target_bir_lowering=False)
    with tile.TileContext(nc) as tc:
        _half_block(tc)
    if waitfix:
        split_excess_waits(nc)
    return nc


def _pack_dir(inputs, di):
    nw = np.asarray(inputs["norm_w"][di], np.float32)
    ipw = np.asarray(inputs["in_proj_w"][di], np.float32) * nw[None, :]
    cw = np.asarray(inputs["conv_w"][di], np.float32)
    cb = np.asarray(inputs["conv_b"][di], np.float32)
    xpw = np.asarray(inputs["x_proj_w"][di], np.float32)
    dtw = np.asarray(inputs["dt_proj_w"][di], np.float32)
    dtb = np.asarray(inputs["dt_proj_b"][di], np.float32)
    Alog = np.asarray(inputs["A_log"][di], np.float32)
    Dp = np.asarray(inputs["D"][di], np.float32)
    opw = np.asarray(inputs["out_proj_w"][di], np.float32)

    wz = np.ascontiguousarray(ipw[DM:2 * DM].T)             # (64, 256)
    ipx = ipw[0:DM]                                         # (256, 64)
    wcomb = np.einsum("dk,dc->kcd", cw, ipx).reshape(4 * C, DM)
    xpw_perm = np.concatenate([xpw[4:36], xpw[0:4]], axis=0)
    wxp = np.ascontiguousarray(xpw_perm.T)                  # (256, 36): B,C,r
    wdt = np.ascontiguousarray(dtw.T)                       # (4, 256)
    A = -np.exp(Alog)                                       # (256, 16)
    ar = A.reshape(32, 8, N).transpose(1, 0, 2)             # (dlo, dhi, n)
    arep = np.broadcast_to(ar[:, None], (8, S, 32, N)).reshape(128, 32 * N)
    wop = np.ascontiguousarray(opw.T)                       # (256, 64)
    return {
        "wz": wz, "wca": wcomb[0:128], "wcb": wcomb[128:256],
        "cbias": np.ascontiguousarray(cb.reshape(2, 128).T),
        "wxa": wxp[0:128], "wxb": wxp[128:256],
        "wdt": wdt, "bdt": np.ascontiguousarray(dtb.reshape(2, 128).T),
        "arep": np.ascontiguousarray(arep),
        "dpar": np.ascontiguousarray(Dp.reshape(2, 128).T),
        "wpa": wop[0:128], "wpb": wop[128:256],
    }


def _launch_inputs(xin_cores, packs, lin_w, lin_b):
    maps = []
    for j in range(NCORES):
        m = {"xin": np.ascontiguousarray(xin_cores[j], np.float32)}
        for d in range(2):
            for k, v in packs[d].items():
                m[f"{k}{d}"] = np.ascontiguousarray(v, np.float32)
        m["linw"] = np.ascontiguousarray(np.asarray(lin_w, np.float32))
        m["linb"] = np.ascontiguousarray(
            np.asarray(lin_b, np.float32).reshape(C, 1))
        maps.append(m)
    return maps


_NC_CACHE = {}
PROFILE = False
LAST_EXEC_NS = []


def _get_nc():
    if "nc" not in _NC_CACHE:
        _NC_CACHE["nc"] = build_program()
    return _NC_CACHE["nc"]


def kernel(**inputs):
    global LAST_EXEC_NS
    from concourse.bass_utils import run_bass_kernel_spmd
    x = np.asarray(inputs["x"], np.float32)  # (1, 64, 128, 128)
    nc = _get_nc()
    LAST_EXEC_NS = []
    kw = {"trace": True} if PROFILE else {}

    packs_t = [_pack_dir(inputs, 0), _pack_dir(inputs, 1)]
    packs_f = [_pack_dir(inputs, 2), _pack_dir(inputs, 3)]

    # ---- launch 1: time scan; seqs = f, core j owns f in [16j, 16j+16) ----
    xin1 = [np.ascontiguousarray(x[0, :, :, 16 * j:16 * j + 16]).reshape(C, T)
            for j in range(NCORES)]
    maps1 = _launch_inputs(xin1, packs_t, inputs["tlin_w"], inputs["tlin_b"])
    res1 = run_bass_kernel_spmd(nc, maps1, list(range(NCORES)), **kw)
    LAST_EXEC_NS.append(res1.exec_time_ns)
    xt = np.concatenate(
        [res1.results[j]["xout"].reshape(C, L, S) for j in range(NCORES)],
        axis=2)  # (64, 128t, 128f)

    # ---- launch 2: freq scan; seqs = t, core j owns t in [16j, 16j+16) ----
    xin2 = [np.ascontiguousarray(
        xt[:, 16 * j:16 * j + 16, :].transpose(0, 2, 1)).reshape(C, T)
        for j in range(NCORES)]
    maps2 = _launch_inputs(xin2, packs_f, inputs["flin_w"], inputs["flin_b"])
    res2 = run_bass_kernel_spmd(nc, maps2, list(range(NCORES)), **kw)
    LAST_EXEC_NS.append(res2.exec_time_ns)

    out = np.empty((1, C, 128, 128), np.float32)
    for j in range(NCORES):
        xf = res2.results[j]["xout"].reshape(C, 128, S)  # (c, f, t_l)
        out[0, :, 16 * j:16 * j + 16, :] = xf.transpose(0, 2, 1)
    return out


# revision 21
# speedup vs baseline: 1.8784x; 1.8784x over previous
"""TF-Mamba block kernel for Trainium2 (8 NeuronCores, SPMD).

The block is two sequential mamba_bidir+linear stages (over time, then over
frequency). Both stages are the same computation on a reshaped token grid,
so ONE bass program ("half-block") is compiled and launched twice; the host
reshards between launches (numpy transpose).

Per launch each core processes 16 sequences x 128 steps for both scan
directions. Token layout on-chip: [channel-partitions, tau*16 + s] where
tau is the scan position and s the sequence. The selective scan runs on the
DVE tensor_tensor_scan instruction with lanes = (s, d_lo) and free dim =
chained (n, tau) segments separated by zeroed reset columns.
"""
import sys

sys.path.insert(0, "/opt/trn_rl_repo")

import numpy as np

import concourse.bass as bass
import concourse.tile as tile
from concourse import mybir
from concourse._compat import with_exitstack

F32 = mybir.dt.float32
BF16 = mybir.dt.bfloat16
AF = mybir.ActivationFunctionType
OP = mybir.AluOpType
EPS = 1e-5

C = 64        # channels
S = 16        # sequences per core per direction
L = 128       # scan length (tau)
T = L * S     # tokens per direction = 2048
DM = 256      # inner model dim (expand*C)
N = 16        # state size
NSEG = L + 1  # scan segment incl. reset column
NCORES = 8

_DIR_PARAMS = [
    ("wz", [C, DM]),       # lhsT for z projection
    ("wca", [128, DM]),    # conv+in_proj fused lhsT, K-tile 0 (k=0,1)
    ("wcb", [128, DM]),    # K-tile 1 (k=2,3)
    ("cbias", [128, 2]),   # conv bias, column per d-tile
    ("wxa", [128, 36]),    # x_proj lhsT K-tile 0
    ("wxb", [128, 36]),
    ("wdt", [4, DM]),
    ("bdt", [128, 2]),
    ("arep", [128, 32 * N]),  # A replicated: [s*8+dlo, dhi*16+n]
    ("dpar", [128, 2]),
    ("wpa", [128, C]),     # out_proj lhsT K-tile 0
    ("wpb", [128, C]),
]


def split_excess_waits(nc, maxw=1):
    """walrus setupSyncWait rejects instructions carrying more than ~2 sync
    waits; hoist the excess onto preceding NoOps on the same engine."""
    ctr = [0]

    def mknop(engine, waits):
        ctr[0] += 1
        nop = mybir.InstNoOp(name=f"waitfix_nop_{ctr[0]}", ins=[], outs=[])
        nop.engine = engine
        nop.sync_info = mybir.SyncInfo(on_wait=list(waits), on_update=[])
        return nop

    for f in nc.m.functions:
        for b in f.blocks:
            out = []
            changed = False
            for inst in b.instructions:
                si = inst.sync_info
                if si is not None and si.on_wait and len(si.on_wait) > maxw:
                    waits = list(si.on_wait)
                    keep, rest = waits[:maxw], waits[maxw:]
                    for k in range(0, len(rest), maxw):
                        out.append(mknop(inst.engine, rest[k:k + maxw]))
                    si.on_wait = keep
                    changed = True
                out.append(inst)
            if changed:
                b.instructions = out


def _seg(t_):
    """[128, 16*129] buffer -> [128, n, tau] view skipping reset columns."""
    return t_[:].rearrange("p (n q) -> p n q", n=N, q=NSEG)[:, :, 1:NSEG]


def _pads(t_):
    return t_[:].rearrange("p (n q) -> p n q", n=N, q=NSEG)[:, :, 0:1]


def _smaj(ap, s=S, t=L):
    """[p, T tau-major] -> [p, s, tau] view."""
    return ap.rearrange("p (t s) -> p t s", t=t, s=s).transpose([0, 2, 1])


def _sview(ap, s=S, t=L):
    """[p, T s-major] -> [p, s, tau] view."""
    return ap.rearrange("p (s t) -> p s t", s=s, t=t)


def _pslq(ap, q):
    """quarter partition window [32q, 32q+32): lanes p = dlo*16 + s."""
    return ap[q * 32:(q + 1) * 32]


SIM_COMPAT = False
AVALS = None


@with_exitstack
def _half_block(ctx, tc):
    nc = tc.nc

    def act_silu(out, in_, bias=0.0, tmp_pool=None):
        if not SIM_COMPAT:
            nc.scalar.activation(out, in_, AF.Silu, bias=bias)
            return
        t1 = tmp_pool.tile(list(in_.shape), F32, tag="silu_t1", name="silu_t1", bufs=1)
        t2 = tmp_pool.tile(list(in_.shape), F32, tag="silu_t2", name="silu_t2", bufs=1)
        nc.scalar.activation(t1[:], in_, AF.Sigmoid, bias=bias)
        nc.scalar.activation(t2[:], in_, AF.Identity, bias=bias)
        nc.vector.tensor_tensor(out, t1[:], t2[:], OP.mult)

    def act_softplus(out, in_, bias, tmp_pool=None):
        # softplus(x+b) = ln(1 + exp(x+b)); Exp and Ln share one ACT
        # table set (natural_log_exp_and_others), as does the scan's Exp.
        t1 = tmp_pool.tile(list(in_.shape), F32, tag="silu_t1",
                           name="silu_t1", bufs=1)
        nc.scalar.activation(t1[:], in_, AF.Exp, bias=bias)
        nc.scalar.activation(out, t1[:], AF.Ln, bias=1.0)
    xin = nc.declare_dram_parameter("xin", [C, T], F32, isOutput=False)
    xout = nc.declare_dram_parameter("xout", [C, T], F32, isOutput=True)
    dirp = []
    for d in range(2):
        dirp.append({
            k: nc.declare_dram_parameter(f"{k}{d}", shp, F32, isOutput=False)
            for k, shp in _DIR_PARAMS
        })
    linw = nc.declare_dram_parameter("linw", [2 * C, C], F32, isOutput=False)
    linb = nc.declare_dram_parameter("linb", [C, 1], F32, isOutput=False)

    wp = ctx.enter_context(tc.tile_pool(name="weights", bufs=1))
    pers = ctx.enter_context(tc.tile_pool(name="pers", bufs=1))
    work = ctx.enter_context(tc.tile_pool(name="work", bufs=1))
    chn = ctx.enter_context(tc.tile_pool(name="chn", bufs=2))
    pp = ctx.enter_context(tc.tile_pool(name="pp", bufs=4, space="PSUM"))
    dram = ctx.enter_context(tc.tile_pool(name="dram", bufs=1, space="DRAM"))

    W = []
    for d in range(2):
        w = {}
        for k, shp in _DIR_PARAMS:
            t_ = wp.tile(shp, F32, tag=f"{k}{d}")
            nc.sync.dma_start(t_[:], dirp[d][k][:])
            w[k] = t_
        W.append(w)
    linw_t = wp.tile([2 * C, C], F32, tag="linw", name="linw")
    nc.sync.dma_start(linw_t[:], linw[:])
    linb_t = wp.tile([C, 1], F32, tag="linb", name="linb")
    nc.sync.dma_start(linb_t[:], linb[:])

    ones_c = wp.tile([C, 1], F32, tag="ones_c", name="ones_c")
    nc.gpsimd.memset(ones_c[:], 1.0)
    ones_1 = wp.tile([1, C], F32, tag="ones_1", name="ones_1")
    nc.gpsimd.memset(ones_1[:], 1.0)
    epsb = wp.tile([1, 1], F32, tag="epsb", name="epsb")
    nc.gpsimd.memset(epsb[:], EPS)

    # ---- shared: load X, rms scale, xn ----
    X = pers.tile([C, T], F32, tag="X", name="X")
    nc.sync.dma_start(X[:], xin[:])
    xsq = work.tile([C, T], F32, tag="scrA", name="scrA")
    nc.scalar.activation(xsq[:], X[:], AF.Square)
    rowstats = work.tile([1, T], F32, tag="scrB", name="scrB")
    for ch in range(4):
        sl = slice(ch * 512, (ch + 1) * 512)
        ps1 = pp.tile([1, 512], F32, tag="ps", name="ps")
        nc.tensor.matmul(ps1[:], ones_c[:], xsq[:, sl], start=True,
                         stop=True)
        nc.scalar.activation(rowstats[:, sl], ps1[:], AF.Sqrt, scale=1.0 / C,
                             bias=epsb[:])
    nc.vector.reciprocal(rowstats[:], rowstats[:])
    rinv = rowstats
    xn = pers.tile([C, T], F32, tag="xn", name="xn")
    for ch in range(4):
        sl = slice(ch * 512, (ch + 1) * 512)
        ps2 = pp.tile([C, 512], F32, tag="ps", name="ps")
        nc.tensor.matmul(ps2[:], ones_1[:], rinv[:, sl], start=True,
                         stop=True)
        nc.vector.tensor_tensor(xn[:, sl], X[:, sl], ps2[:], OP.mult)

    cat = pers.tile([2 * C, T], F32, tag="cat", name="cat")
    xn3 = xn[:].rearrange("p (t s) -> p t s", t=L, s=S)

    for d in range(2):
        fwd = (d == 0)
        w = W[d]
        # ---- conv stacks (shift in tau; bwd reads xn reversed in tau) ----
        stks = [work.tile([128, T], F32, tag=f"stk{i}", name=f"stk{i}") for i in range(2)]
        for k in range(4):
            dst = stks[k // 2][(k % 2) * C:(k % 2) * C + C, :]
            dst3 = dst.rearrange("p (t s) -> p t s", t=L, s=S)
            pad = 3 - k
            src3 = xn3 if fwd else xn3[:, ::-1, :]
            if pad > 0:
                nc.gpsimd.memset(dst3[:, 0:pad, :], 0.0)
                nc.vector.tensor_copy(dst3[:, pad:L, :], src3[:, 0:L - pad, :])
            else:
                nc.vector.tensor_copy(dst3, src3)

        # ---- z projection + silu (natural tau order for both dirs) ----
        zsil = work.tile([128, 2 * T], BF16, tag="zsil", name="zsil")
        for mt in range(2):
            for ch in range(4):
                sl = slice(ch * 512, (ch + 1) * 512)
                osl = slice(mt * T + ch * 512, mt * T + (ch + 1) * 512)
                ps = pp.tile([128, 512], F32, tag="ps", name="ps")
                nc.tensor.matmul(ps[:],
                                 w["wz"][:, mt * 128:mt * 128 + 128],
                                 xn[:, sl], start=True, stop=True)
                act_silu(zsil[:, osl], ps[:], tmp_pool=pp)

        # ---- fused conv + in_proj-xc + bias + silu ----
        xc = work.tile([128, 2 * T], F32, tag="xc", name="xc")
        for mt in range(2):
            msl = slice(mt * 128, mt * 128 + 128)
            for ch in range(4):
                sl = slice(ch * 512, (ch + 1) * 512)
                osl = slice(mt * T + ch * 512, mt * T + (ch + 1) * 512)
                ps = pp.tile([128, 512], F32, tag="ps", name="ps")
                nc.tensor.matmul(ps[:], w["wca"][:, msl], stks[0][:, sl],
                                 start=True, stop=False)
                nc.tensor.matmul(ps[:], w["wcb"][:, msl], stks[1][:, sl],
                                 start=False, stop=True)
                act_silu(xc[:, osl], ps[:], bias=w["cbias"][:, mt:mt + 1],
                         tmp_pool=pp)

        # ---- x_proj -> dbl [36, T] ----
        dbl = work.tile([36, T], F32, tag="dbl_brep", name="dbl_brep")
        for ch in range(4):
            sl = slice(ch * 512, (ch + 1) * 512)
            ps = pp.tile([36, 512], F32, tag="ps", name="ps")
            nc.tensor.matmul(ps[:], w["wxa"][:],
                             xc[:, sl], start=True, stop=False)
            nc.tensor.matmul(ps[:], w["wxb"][:],
                             xc[:, T + ch * 512:T + (ch + 1) * 512],
                             start=False, stop=True)
            nc.scalar.copy(dbl[:, sl], ps[:])

        # ---- dt = softplus(dt_proj + bias), written s-major ----
        dblr = wp.tile([4, T], F32, tag="dblr", name="dblr")
        nc.vector.tensor_copy(dblr[:], dbl[32:36, :])
        dt_mm = work.tile([128, 2 * T], BF16, tag="dtmm", name="dtmm")
        for mt in range(2):
            msl = slice(mt * 128, mt * 128 + 128)
            for ch in range(4):
                ps = pp.tile([128, 512], F32, tag="ps", name="ps")
                nc.tensor.matmul(ps[:], w["wdt"][:, msl],
                                 dblr[0:4, ch * 512:(ch + 1) * 512],
                                 start=True, stop=True)
                psr = ps[:].rearrange("p (t s) -> p t s", t=32, s=S)
                dtv = dt_mm[:, mt * T:(mt + 1) * T]
                dtv = dtv.rearrange("p (s t) -> p s t", s=S, t=L)
                dtv = dtv[:, :, ch * 32:(ch + 1) * 32].transpose([0, 2, 1])
                act_softplus(dtv, psr, w["bdt"][:, mt:mt + 1], tmp_pool=pp)

        # ---- u = dt * xc (s-major) ----
        u_mm = work.tile([128, 2 * T], BF16, tag="umm", name="umm")
        for mt in range(2):
            nc.vector.tensor_tensor(
                _sview(u_mm[:, mt * T:(mt + 1) * T]),
                _sview(dt_mm[:, mt * T:(mt + 1) * T]),
                _smaj(xc[:, mt * T:(mt + 1) * T]), OP.mult)

        # ---- B/C staging s-major [32, T] ----
        bcs = work.tile([32, T], BF16, tag="bcs_crep", name="bcs_crep")
        nc.vector.tensor_copy(_sview(bcs[:]), _smaj(dbl[0:32, :]))

        # ---- shuffles through DRAM ----
        dt_d = dram.tile([DM, T], BF16, tag="dt_d", name="dt_d")
        u_d = dram.tile([DM, T], BF16, tag="u_d", name="u_d")
        bc_d = dram.tile([128, 32 * L], BF16, tag="bc_d", name="bc_d")
        for mt in range(2):
            msl = slice(mt * 128, (mt + 1) * 128)
            nc.sync.dma_start(dt_d[msl, :], dt_mm[:, mt * T:(mt + 1) * T])
            nc.sync.dma_start(u_d[msl, :], u_mm[:, mt * T:(mt + 1) * T])
        # bcs rows (n') scatter to bc_d[dlo*16+s][n'][tau], replicated
        # over dlo so the scan-land read is a plain [128, :] copy
        for dlo in range(8):
            nc.sync.dma_start(
                bc_d[dlo * S:(dlo + 1) * S, :].rearrange(
                    "s (n t) -> n s t", n=32, t=L), bcs[:])

        dtS = work.tile([128, 32 * L], BF16, tag="stk0", name="dtS")
        uS = work.tile([128, 32 * L], BF16, tag="stk1", name="uS")
        brep = work.tile([128, N * L], BF16, tag="dbl_brep", name="brep")
        crep = work.tile([128, N * L], BF16, tag="bcs_crep", name="crep")
        dtdv = dt_d[:].rearrange("(dhi dlo) f -> dhi dlo f", dhi=32, dlo=8)
        udv = u_d[:].rearrange("(dhi dlo) f -> dhi dlo f", dhi=32, dlo=8)
        for dhi in range(32):
            fsl = slice(dhi * L, (dhi + 1) * L)
            nc.sync.dma_start(dtS[:, fsl],
                              dtdv[dhi].rearrange("dlo (s t) -> (dlo s) t",
                                                  s=S, t=L))
            nc.sync.dma_start(uS[:, fsl],
                              udv[dhi].rearrange("dlo (s t) -> (dlo s) t",
                                                 s=S, t=L))
        nc.sync.dma_start(brep[:], bc_d[:, 0:N * L])
        nc.sync.dma_start(crep[:], bc_d[:, N * L:2 * N * L])
        brep3 = brep[:].rearrange("p (n t) -> p n t", n=N, t=L)
        crep3 = crep[:].rearrange("p (n t) -> p n t", n=N, t=L)

        # ---- scan over d_hi chunks ----
        yS = work.tile([128, 32 * L], F32, tag="scrA", name="scrA")
        for g in range(32):
            gsl = slice(g * L, (g + 1) * L)
            dA = chn.tile([128, N * NSEG], F32, tag="dA", name="dA")
            ub = chn.tile([128, N * NSEG], BF16, tag="ub", name="ub")
            h = chn.tile([128, N * NSEG], BF16, tag="h", name="h")
            yt = chn.tile([128, N * L], BF16, tag="yt", name="yt", bufs=1)
            nc.gpsimd.memset(_pads(dA), 0.0)
            nc.gpsimd.memset(_pads(ub), 0.0)
            if AVALS is not None:
                # A is d-independent: dA[n] = exp(A_n * dt), scale baked
                for n in range(N):
                    nc.scalar.activation(
                        dA[:, n * NSEG + 1:(n + 1) * NSEG], dtS[:, gsl],
                        AF.Exp, scale=float(AVALS[n]))
            else:
                dts_b = dtS[:, gsl].unsqueeze(1).broadcast_to([128, N, L])
                ar_b = w["arep"][:, g * N:(g + 1) * N].unsqueeze(2)
                ar_b = ar_b.broadcast_to([128, N, L])
                nc.vector.tensor_tensor(_seg(h), dts_b, ar_b, OP.mult)
                nc.scalar.activation(_seg(dA), _seg(h), AF.Exp)
            us_b = uS[:, gsl].unsqueeze(1).broadcast_to([128, N, L])
            nc.gpsimd.tensor_tensor(_seg(ub), us_b, brep3, OP.mult)
            nc.vector.tensor_tensor_scan(h[:], dA[:], ub[:], 0.0,
                                         OP.mult, OP.add)
            ytv = yt[:].rearrange("p (n t) -> p n t", n=N, t=L)
            nc.vector.tensor_tensor(ytv, _seg(h), crep3, OP.mult)
            nc.vector.tensor_reduce(yS[:, gsl], ytv.transpose([0, 2, 1]),
                                    mybir.AxisListType.X, OP.add)

        # ---- y back to [d, (s,t)] ----
        y_d = dram.tile([DM, T], F32, tag="y_d", name="y_d")
        ydv = y_d[:].rearrange("(dhi dlo) f -> dhi dlo f", dhi=32, dlo=8)
        for dhi in range(32):
            nc.sync.dma_start(
                ydv[dhi].rearrange("dlo (s t) -> (dlo s) t", s=S, t=L),
                yS[:, dhi * L:(dhi + 1) * L])
        y_mm = work.tile([128, 2 * T], F32, tag="ymm", name="ymm")
        for mt in range(2):
            nc.sync.dma_start(y_mm[:, mt * T:(mt + 1) * T],
                              y_d[mt * 128:(mt + 1) * 128, :])

        # ---- gating: g2 = (y + D*xc) * silu(z), all s-major ----
        g2 = work.tile([128, 2 * T], F32, tag="g2", name="g2")
        for mt in range(2):
            g1 = work.tile([128, T], F32, tag="scrB", name="scrB")
            nc.vector.scalar_tensor_tensor(
                _sview(g1[:]), _smaj(xc[:, mt * T:(mt + 1) * T]),
                w["dpar"][:, mt:mt + 1],
                _sview(y_mm[:, mt * T:(mt + 1) * T]), OP.mult, OP.add)
            zv = _smaj(zsil[:, mt * T:(mt + 1) * T])
            if not fwd:
                zv = zv[:, :, ::-1]
            nc.vector.tensor_tensor(_sview(g2[:, mt * T:(mt + 1) * T]),
                                    _sview(g1[:]), zv, OP.mult)

        # ---- out_proj + residual into cat rows ----
        catd = _sview(cat[d * C:d * C + C, :])
        Xv = _smaj(X[:])
        for ch in range(4):
            sl = slice(ch * 512, (ch + 1) * 512)
            ssl = slice(ch * 4, (ch + 1) * 4)
            ps = pp.tile([C, 512], F32, tag="ps", name="ps")
            nc.tensor.matmul(ps[:], w["wpa"][:], g2[:, sl],
                             start=True, stop=False)
            nc.tensor.matmul(ps[:], w["wpb"][:],
                             g2[:, T + ch * 512:T + (ch + 1) * 512],
                             start=False, stop=True)
            psv = ps[:].rearrange("p (s t) -> p s t", s=4, t=L)
            if fwd:
                nc.vector.tensor_tensor(catd[:, ssl, :], psv, Xv[:, ssl, :],
                                        OP.add)
            else:
                nc.vector.tensor_tensor(catd[:, ssl, :], psv[:, :, ::-1],
                                        Xv[:, ssl, :], OP.add)

    # ---- lin + bias + residual -> xout (cat is s-major natural-t) ----
    xo = work.tile([C, T], F32, tag="scrB", name="scrB")
    for ch in range(4):
        sl = slice(ch * 512, (ch + 1) * 512)
        ssl = slice(ch * 4, (ch + 1) * 4)
        ps = pp.tile([C, 512], F32, tag="ps", name="ps")
        nc.tensor.matmul(ps[:], linw_t[:], cat[:, sl], start=True,
                         stop=True)
        psv = ps[:].rearrange("p (s t) -> p s t", s=4, t=L)
        nc.vector.scalar_tensor_tensor(_smaj(xo[:])[:, ssl, :], psv,
                                       linb_t[:], _smaj(X[:])[:, ssl, :],
                                       OP.add, OP.add)
    nc.sync.dma_start(xout[:], xo[:])


def build_program(waitfix=True, sim_compat=False, avals=None):
    global SIM_COMPAT, AVALS
    SIM_COMPAT = sim_compat
    AVALS = avals
    nc = bass.Bass(target_bir_lowering=False)
    with tile.TileContext(nc) as tc:
        _half_block(tc)
    if waitfix:
        split_excess_waits(nc)
    return nc


def _pack_dir(inputs, di):
    nw = np.asarray(inputs["norm_w"][di], np.float32)
    ipw = np.asarray(inputs["in_proj_w"][di], np.float32) * nw[None, :]
    cw = np.asarray(inputs["conv_w"][di], np.float32)
    cb = np.asarray(inputs["conv_b"][di], np.float32)
    xpw = np.asarray(inputs["x_proj_w"][di], np.float32)
    dtw = np.asarray(inputs["dt_proj_w"][di], np.float32)
    dtb = np.asarray(inputs["dt_proj_b"][di], np.float32)
    Alog = np.asarray(inputs["A_log"][di], np.float32)
    Dp = np.asarray(inputs["D"][di], np.float32)
    opw = np.asarray(inputs["out_proj_w"][di], np.float32)

    wz = np.ascontiguousarray(ipw[DM:2 * DM].T)             # (64, 256)
    ipx = ipw[0:DM]                                         # (256, 64)
    wcomb = np.einsum("dk,dc->kcd", cw, ipx).reshape(4 * C, DM)
    xpw_perm = np.concatenate([xpw[4:36], xpw[0:4]], axis=0)
    wxp = np.ascontiguousarray(xpw_perm.T)                  # (256, 36): B,C,r
    wdt = np.ascontiguousarray(dtw.T)                       # (4, 256)
    A = -np.exp(Alog)                                       # (256, 16)
    ar = A.reshape(32, 8, N).transpose(1, 0, 2)             # (dlo, dhi, n)
    arep = np.broadcast_to(ar[:, None], (8, S, 32, N)).reshape(128, 32 * N)
    wop = np.ascontiguousarray(opw.T)                       # (256, 64)
    return {
        "wz": wz, "wca": wcomb[0:128], "wcb": wcomb[128:256],
        "cbias": np.ascontiguousarray(cb.reshape(2, 128).T),
        "wxa": wxp[0:128], "wxb": wxp[128:256],
        "wdt": wdt, "bdt": np.ascontiguousarray(dtb.reshape(2, 128).T),
        "arep": np.ascontiguousarray(arep),
        "dpar": np.ascontiguousarray(Dp.reshape(2, 128).T),
        "wpa": wop[0:128], "wpb": wop[128:256],
    }


def _launch_inputs(xin_cores, packs, lin_w, lin_b):
    maps = []
    for j in range(NCORES):
        m = {"xin": np.ascontiguousarray(xin_cores[j], np.float32)}
        for d in range(2):
            for k, v in packs[d].items():
                m[f"{k}{d}"] = np.ascontiguousarray(v, np.float32)
        m["linw"] = np.ascontiguousarray(np.asarray(lin_w, np.float32))
        m["linb"] = np.ascontiguousarray(
            np.asarray(lin_b, np.float32).reshape(C, 1))
        maps.append(m)
    return maps


_NC_CACHE = {}
PROFILE = False
LAST_EXEC_NS = []


def _get_nc(avals_key=None):
    key = ("nc", avals_key)
    if key not in _NC_CACHE:
        _NC_CACHE[key] = build_program(
            avals=None if avals_key is None else list(avals_key))
    return _NC_CACHE[key]


def _fast_avals(inputs):
    """If A = -exp(A_log) is d-independent and shared by all 4 dirs,
    return the 16 scalars for baking into ACT immediates."""
    A = -np.exp(np.asarray(inputs["A_log"], np.float64))  # (4, 256, 16)
    ref0 = A[0, 0]
    if np.allclose(A, ref0[None, None, :], rtol=1e-6, atol=0):
        return tuple(float(v) for v in ref0)
    return None


def kernel(**inputs):
    global LAST_EXEC_NS
    from concourse.bass_utils import run_bass_kernel_spmd
    x = np.asarray(inputs["x"], np.float32)  # (1, 64, 128, 128)
    nc = _get_nc(_fast_avals(inputs))
    LAST_EXEC_NS = []
    kw = {"trace": True} if PROFILE else {}

    packs_t = [_pack_dir(inputs, 0), _pack_dir(inputs, 1)]
    packs_f = [_pack_dir(inputs, 2), _pack_dir(inputs, 3)]

    # ---- launch 1: time scan; seqs = f, core j owns f in [16j, 16j+16) ----
    xin1 = [np.ascontiguousarray(x[0, :, :, 16 * j:16 * j + 16]).reshape(C, T)
            for j in range(NCORES)]
    maps1 = _launch_inputs(xin1, packs_t, inputs["tlin_w"], inputs["tlin_b"])
    res1 = run_bass_kernel_spmd(nc, maps1, list(range(NCORES)), **kw)
    LAST_EXEC_NS.append(res1.exec_time_ns)
    xt = np.concatenate(
        [res1.results[j]["xout"].reshape(C, L, S) for j in range(NCORES)],
        axis=2)  # (64, 128t, 128f)

    # ---- launch 2: freq scan; seqs = t, core j owns t in [16j, 16j+16) ----
    xin2 = [np.ascontiguousarray(
        xt[:, 16 * j:16 * j + 16, :].transpose(0, 2, 1)).reshape(C, T)
        for j in range(NCORES)]
    maps2 = _launch_inputs(xin2, packs_f, inputs["flin_w"], inputs["flin_b"])
    res2 = run_bass_kernel_spmd(nc, maps2, list(range(NCORES)), **kw)
    LAST_EXEC_NS.append(res2.exec_time_ns)

    out = np.empty((1, C, 128, 128), np.float32)
    for j in range(NCORES):
        xf = res2.results[j]["xout"].reshape(C, 128, S)  # (c, f, t_l)
        out[0, :, 16 * j:16 * j + 16, :] = xf.transpose(0, 2, 1)
    return out
